# revision 29
# baseline (speedup 1.0000x reference)
"""Bass/Trainium2 kernel for the DGPE relaxation RHS on a 192^3 periodic lattice.

v2: the linear 6-neighbor stencil runs on the (otherwise idle) Tensor engine
as shifted-view matmuls against diagonal stationaries (J*I for the 4 in-plane
neighbors, J*anis*I for the z pair) accumulated in PSUM fp32; ScalarE
evacuates PSUM->SBUF (bf16) and computes the squares; VectorE does only the
12 irreducible tensor-tensor ops in bf16 (2x packed mode).  All HBM traffic
is bf16.  Lattice sharded along axis 0 across 8 cores (24 planes each);
within a core partition = (k-block, j-block) = 8 x 16, each partition a
(24 x 12 x 24) brick with j-halo 1 and k-halo 2 (even offsets keep the DVE
center views 4B-aligned for 2x mode).
"""

import numpy as np
import ml_dtypes

BF16 = ml_dtypes.bfloat16

L = 192
N = L ** 3
NCORES = 8
CH = L // NCORES            # 24 planes (axis 0) per core
KH, JB = 8, 16              # partition grid: p = kh*JB + jb
JW = L // JB                # 12 j's per partition
KW = L // KH                # 24 k's per partition
IH = CH + 2                 # 26 planes incl. axis-0 halo
FJ = JW + 2                 # 14 incl. j halo (1 each side)
FK = KW + 4                 # 28 incl. k halo (2 each side)
PLF = FJ * FK               # padded plane free size (392)
FIN = IH * PLF              # 10192
PF = JW * KW                # compact plane free size (288)
FOUT = CH * PF              # 6912
T = 8                       # planes per compute tile
NT = CH // T
TF = T * PF                 # 2304

_STATE = {}


# ---------------------------------------------------------------- host side

def _shard_halo(v3):
    """(192,192,192) fp32 -> (8, 128, FIN) bf16 per-core halo'd bricks."""
    vp = np.pad(v3, ((1, 1), (1, 1), (2, 2)), mode="wrap").astype(BF16)
    s0, s1, s2 = vp.strides
    v = np.lib.stride_tricks.as_strided(
        vp,
        shape=(NCORES, KH, JB, IH, FJ, FK),
        strides=(CH * s0, KW * s2, JW * s1, s0, s1, s2),
    )
    return np.ascontiguousarray(v).reshape(NCORES, 128, FIN)


def _shard_compact(v3):
    """(192,192,192) -> (8, 128, CH, PF) per-core compact bf16 images."""
    v3 = v3.astype(BF16)
    s0, s1, s2 = v3.strides
    v = np.lib.stride_tricks.as_strided(
        v3,
        shape=(NCORES, KH, JB, CH, JW, KW),
        strides=(CH * s0, KW * s2, JW * s1, s0, s1, s2),
    )
    return np.ascontiguousarray(v).reshape(NCORES, 128, CH, PF)


def _unshard_compact(per_core):
    """(8, 128, CH*PF) fp32 -> (192,192,192)."""
    out3 = np.empty((L, L, L), np.float32)
    s0, s1, s2 = out3.strides
    w = np.lib.stride_tricks.as_strided(
        out3,
        shape=(NCORES, KH, JB, CH, JW, KW),
        strides=(CH * s0, KW * s2, JW * s1, s0, s1, s2),
    )
    w[:] = per_core.reshape(NCORES, KH, JB, CH, JW, KW)
    return out3


def _is_const(a):
    a = np.asarray(a)
    return bool(a.size) and bool(np.all(a == a.flat[0]))


def _rolls_ok(nn_idx_1, nn_idx_2, nn_idy_1, nn_idy_2, nn_idz_1, nn_idz_2):
    """Spot-check that the index arrays are the periodic roll stencil."""
    rng = np.random.default_rng(12345)
    f = rng.integers(0, N, size=4096)
    i, r = np.divmod(f, L * L)
    j, k = np.divmod(r, L)

    def flat(ii, jj, kk):
        return (ii % L) * L * L + (jj % L) * L + (kk % L)

    checks = [
        (nn_idx_1, flat(i - 1, j, k)), (nn_idx_2, flat(i + 1, j, k)),
        (nn_idy_1, flat(i, j - 1, k)), (nn_idy_2, flat(i, j + 1, k)),
        (nn_idz_1, flat(i, j, k - 1)), (nn_idz_2, flat(i, j, k + 1)),
    ]
    for arr, want in checks:
        if not np.array_equal(np.asarray(arr)[f], want):
            return False
    return True


def _numpy_fallback(y, J, anisotropy, gamma, h_dis_x, h_dis_y, beta,
                    e_disorder, idx):
    """Exact reference math in numpy (used only if structure checks fail)."""
    x, p = y[:N], y[N:]

    def stencil(v):
        return J * (v[idx[0]] + v[idx[1]] + v[idx[2]] + v[idx[3]]
                    + anisotropy * (v[idx[4]] + v[idx[5]]))

    xL = stencil(x)
    yL = stencil(p)
    r2 = x * x + p * p
    cross = xL * p - yL * x
    dx = gamma * p * cross + e_disorder * p - yL + h_dis_y + beta * r2 * p
    dp = -gamma * x * cross - e_disorder * x + xL - h_dis_x - beta * r2 * x
    return np.concatenate([dx, dp]).astype(np.float32)


# -------------------------------------------------------------- device side

def _build_nc():
    from concourse import bacc
    import concourse.mybir as mybir
    from concourse.mybir import AluOpType as Op
    from concourse.tile import TileContext, add_dep_helper

    ActF = mybir.ActivationFunctionType
    f32 = mybir.dt.float32
    bf16 = mybir.dt.bfloat16

    nc = bacc.Bacc("TRN2", target_bir_lowering=False, debug=False,
                   enable_asserts=False, num_devices=NCORES)
    x_in = nc.dram_tensor("x_in", [128, FIN], bf16, kind="ExternalInput").ap()
    p_in = nc.dram_tensor("p_in", [128, FIN], bf16, kind="ExternalInput").ap()
    # packed per-tile coefficients: [e_disorder | h_dis_x | h_dis_y]
    cf_in = nc.dram_tensor("cf_in", [128, 3, FOUT], bf16, kind="ExternalInput").ap()
    w_in = nc.dram_tensor("w_in", [128, 2, 128], bf16, kind="ExternalInput").ap()
    cst_in = nc.dram_tensor("cst_in", [128, 4], f32, kind="ExternalInput").ap()
    dx_out = nc.dram_tensor("dx_out", [128, FOUT], bf16, kind="ExternalOutput").ap()
    dp_out = nc.dram_tensor("dp_out", [128, FOUT], bf16, kind="ExternalOutput").ap()

    with TileContext(nc) as tc:
        with (
            tc.tile_pool(name="persist", bufs=1) as pers,
            tc.tile_pool(name="coef", bufs=2) as cp,
            tc.tile_pool(name="ev", bufs=2) as ep,
            tc.tile_pool(name="sq", bufs=2) as qp,
            tc.tile_pool(name="tmp", bufs=2) as tp,
            tc.tile_pool(name="outs", bufs=2) as op_,
            tc.psum_pool(name="ps", bufs=4) as pp,
        ):
            xt = pers.tile([128, FIN], bf16, name="xt")
            pt = pers.tile([128, FIN], bf16, name="pt")
            wj = pers.tile([128, 128], bf16, name="wj")
            wa = pers.tile([128, 128], bf16, name="wa")
            cst = pers.tile([128, 4], f32, name="cst")
            GAM = cst[:, 0:1]    # gamma
            SQB = cst[:, 2:3]    # sqrt(beta/gamma)

            # tiny weight/const DMAs first (the PE warmup waits only on wj),
            # then progressive 2-plane x/p chunks that the PE chases
            nc.sync.dma_start(wj[:], w_in[:, 0])
            nc.sync.dma_start(wa[:], w_in[:, 1])
            nc.sync.dma_start(cst[:], cst_in)

            # ~4us of throwaway matmuls so the PE HAM un-throttles (1.2 ->
            # 2.4 GHz) before the real stencil work arrives; overlaps the
            # initial x/p DMA
            pw = pp.tile([128, 512], f32, tag="ps", name="warm")
            for _ in range(36):
                nc.tensor.matmul(pw[:, :128], wj[:], wj[:],
                                 start=True, stop=True)

            def v4(img):
                return img[:].rearrange("q (i j k) -> q i j k",
                                        i=IH, j=FJ, k=FK)

            xv, pv = v4(xt), v4(pt)

            # variable tile sizes: a small-to-large ladder shortens the PE
            # ramp before the Vector engine has its first full tile, while
            # keeping the PE comfortably ahead of the DVE afterwards
            segs = [(0, 4), (4, 4), (8, 8), (16, 8)]
            # x/p plane chunks emitted right before the segment needing them
            seg_chunks = [[(0, 4), (4, 6)], [(6, 8), (8, 10)],
                          [(10, 18)], [(18, 26)]]

            for s, (o, Tt) in enumerate(segs):
                TFt = Tt * PF
                for (a, b) in seg_chunks[s]:
                    nc.sync.dma_start(xt[:, a * PLF:b * PLF],
                                      x_in[:, a * PLF:b * PLF])
                    nc.sync.dma_start(pt[:, a * PLF:b * PLF],
                                      p_in[:, a * PLF:b * PLF])
                if s == 0:
                    # bridge batch: keep the PE busy (HAM warm) while the
                    # first x/p chunks are still in flight
                    for _ in range(6):
                        nc.tensor.matmul(pw[:, :128], wj[:], wj[:],
                                         start=True, stop=True)
                ct = cp.tile([128, 3, TF], bf16, tag="ct", name=f"ct{s}")
                for c in range(3):
                    nc.sync.dma_start(ct[:, c, :TFt],
                                      cf_in[:, c, o * PF:o * PF + TFt])
                ed, hx, hy = ct[:, 0, :TFt], ct[:, 1, :TFt], ct[:, 2, :TFt]

                def vt(tile):
                    return tile[:, :TFt].rearrange(
                        "q (i j k) -> q i j k", i=Tt, j=JW, k=KW)

                # center views (even offsets -> DVE 2x packed mode)
                i0 = 1 + o
                xc = xv[:, i0:i0 + Tt, 1:1 + JW, 2:2 + KW]
                pc = pv[:, i0:i0 + Tt, 1:1 + JW, 2:2 + KW]

                # ---- ScalarE: scaled squares q = (beta/gamma) * v^2
                q1 = qp.tile([128, TF], bf16, tag="q1", name=f"q1_{s}")
                q2 = qp.tile([128, TF], bf16, tag="q2", name=f"q2_{s}")
                nc.scalar.activation(vt(q1), xc, ActF.Square, scale=SQB)
                nc.scalar.activation(vt(q2), pc, ActF.Square, scale=SQB)

                # ---- TensorE: stencil XL/YL per output plane, PSUM accumulate.
                # 2-plane groups, matmuls grouped by stationary
                XLt = ep.tile([128, TF], bf16, tag="XL", name=f"XL{s}")
                YLt = ep.tile([128, TF], bf16, tag="YL", name=f"YL{s}")
                for g in range(Tt // 2):
                    # one 2-bank psum tile per field holds both planes of the
                    # group -> half the allocations/evacs (shorter sem tail)
                    chunks = []
                    for (fv, dst, nm) in ((xv, XLt, "x"), (pv, YLt, "y")):
                        acc = pp.tile([128, 1024], f32, tag="ps",
                                      name=f"ps{s}_{g}_{nm}")
                        chunks.append((acc, fv, dst))
                    for (acc, fv, dst) in chunks:
                        for h, q in enumerate((2 * g, 2 * g + 1)):
                            pso = acc[:, h * 512:h * 512 + PF]
                            pi = i0 + q
                            nc.tensor.matmul(pso, wj[:], fv[:, pi - 1:pi, 1:1 + JW, 2:2 + KW],
                                             start=True, stop=False, skip_group_check=True)
                            nc.tensor.matmul(pso, wj[:], fv[:, pi + 1:pi + 2, 1:1 + JW, 2:2 + KW],
                                             start=False, stop=False, skip_group_check=True)
                            nc.tensor.matmul(pso, wj[:], fv[:, pi:pi + 1, 0:JW, 2:2 + KW],
                                             start=False, stop=False, skip_group_check=True)
                            nc.tensor.matmul(pso, wj[:], fv[:, pi:pi + 1, 2:2 + JW, 2:2 + KW],
                                             start=False, stop=False, skip_group_check=True)
                    for (acc, fv, dst) in chunks:
                        for h, q in enumerate((2 * g, 2 * g + 1)):
                            pso = acc[:, h * 512:h * 512 + PF]
                            pi = i0 + q
                            nc.tensor.matmul(pso, wa[:], fv[:, pi:pi + 1, 1:1 + JW, 1:1 + KW],
                                             start=False, stop=False, skip_group_check=True)
                            nc.tensor.matmul(pso, wa[:], fv[:, pi:pi + 1, 1:1 + JW, 3:3 + KW],
                                             start=False, stop=True, skip_group_check=True)
                    for (acc, fv, dst) in chunks:
                        # ScalarE evacuation PSUM -> SBUF (bf16), both planes
                        # in one strided op
                        pv2 = acc[:].rearrange("q (g r) -> q g r", g=2, r=512)[:, :, :PF]
                        dv2 = dst[:, 2 * g * PF:(2 * g + 2) * PF].rearrange(
                            "q (g r) -> q g r", g=2, r=PF)
                        nc.scalar.copy(dv2, pv2)

                # ---- VectorE: 13 bf16 ops (12 tensor-tensor @2x + 1 @4x)
                r2 = tp.tile([128, TF], bf16, tag="r2", name=f"r2_{s}")
                c1 = tp.tile([128, TF], bf16, tag="c1", name=f"c1_{s}")
                c2 = tp.tile([128, TF], bf16, tag="c2", name=f"c2_{s}")
                u1 = tp.tile([128, TF], bf16, tag="u1", name=f"u1_{s}")
                u2 = tp.tile([128, TF], bf16, tag="u2", name=f"u2_{s}")
                nc.vector.tensor_add(r2[:, :TFt], q1[:, :TFt], q2[:, :TFt])
                nc.vector.tensor_mul(vt(c1), vt(YLt), xc)            # c1 = YL*x
                nc.vector.tensor_mul(vt(c2), vt(XLt), pc)            # c2 = XL*p
                nc.vector.tensor_sub(c2[:, :TFt], c2[:, :TFt], c1[:, :TFt])
                nc.vector.tensor_add(c2[:, :TFt], c2[:, :TFt], ed)
                nc.vector.tensor_add(c2[:, :TFt], c2[:, :TFt], r2[:, :TFt])
                nc.vector.tensor_scalar_mul(c2[:, :TFt], c2[:, :TFt], GAM)

                dxo = op_.tile([128, TF], bf16, tag="dxo", name=f"dxo{s}")
                dpo = op_.tile([128, TF], bf16, tag="dpo", name=f"dpo{s}")
                f0 = o * PF
                nc.vector.tensor_mul(vt(dxo), vt(c2), pc)            # t1 = p*s2
                nc.vector.tensor_sub(u1[:, :TFt], hy, YLt[:, :TFt])  # u1
                nc.vector.tensor_add(dxo[:, :TFt], dxo[:, :TFt], u1[:, :TFt])
                nc.sync.dma_start(dx_out[:, f0:f0 + TFt], dxo[:, :TFt])
                nc.vector.tensor_sub(u2[:, :TFt], XLt[:, :TFt], hx)  # u2
                nc.vector.tensor_mul(vt(dpo), vt(c2), xc)            # t2 = x*s2
                if s == len(segs) - 1:
                    # split the last store so the final DMA overlaps compute
                    Hh = TFt // 2
                    for a, b in ((0, Hh), (Hh, TFt)):
                        nc.vector.tensor_sub(dpo[:, a:b], u2[:, a:b],
                                             dpo[:, a:b])
                        nc.sync.dma_start(dp_out[:, f0 + a:f0 + b],
                                          dpo[:, a:b])
                else:
                    nc.vector.tensor_sub(dpo[:, :TFt], u2[:, :TFt],
                                         dpo[:, :TFt])
                    nc.sync.dma_start(dp_out[:, f0:f0 + TFt], dpo[:, :TFt])

    nc.compile()
    return nc


def _get_nc():
    if "nc" not in _STATE:
        _STATE["nc"] = _build_nc()
    return _STATE["nc"]


def _run(in_maps, trace=False, trace_cores=None):
    from concourse.bass_utils import run_bass_kernel_spmd
    if trace:
        # the agent image's antenv lacks axon_hooks; wire the NTFF hook
        import sys as _sys
        import types as _types
        if "antenv.axon_hooks" not in _sys.modules:
            try:
                import trn_agent_boot.trn_boot as _tb
                _hook = _tb._ntff_profile_via_ctypes('/opt/axon/libaxon_pjrt.so')
                _mod = _types.ModuleType("antenv.axon_hooks")
                _mod.get_axon_ntff_profile_hook = lambda: _hook
                _sys.modules["antenv.axon_hooks"] = _mod
            except Exception:
                pass
    return run_bass_kernel_spmd(
        _get_nc(), in_maps, core_ids=list(range(NCORES)),
        trace=trace, trace_cores=trace_cores,
    )


def prepare_in_maps(y, anis_v, gamma_v, beta_v, j_v, h_dis_x, h_dis_y,
                    e_disorder):
    """Host-side sharding: build the 8 per-core input maps."""
    x3 = np.ascontiguousarray(y[:N], np.float32).reshape(L, L, L)
    p3 = np.ascontiguousarray(y[N:], np.float32).reshape(L, L, L)
    xs = _shard_halo(x3)
    ps = _shard_halo(p3)
    eds = _shard_compact((np.asarray(e_disorder, np.float32) / gamma_v)
                         .reshape(L, L, L))
    hxs = _shard_compact(np.asarray(h_dis_x, np.float32).reshape(L, L, L))
    hys = _shard_compact(np.asarray(h_dis_y, np.float32).reshape(L, L, L))
    # coefficient block: [3, FOUT] = [e_dis/gamma | h_dis_x | h_dis_y]
    cf = np.stack([eds.reshape(NCORES, 128, FOUT),
                   hxs.reshape(NCORES, 128, FOUT),
                   hys.reshape(NCORES, 128, FOUT)], axis=2)
    cf = np.ascontiguousarray(cf)          # (8, 128, 3, FOUT)
    w = np.zeros((128, 2, 128), np.float32)
    w[:, 0][np.arange(128), np.arange(128)] = j_v
    w[:, 1][np.arange(128), np.arange(128)] = j_v * anis_v
    w = w.astype(BF16)
    cst = np.zeros((128, 4), np.float32)
    cst[:, 0] = gamma_v
    cst[:, 1] = beta_v
    cst[:, 2] = np.sqrt(beta_v / gamma_v)
    return [
        {"x_in": xs[c], "p_in": ps[c], "cf_in": cf[c], "w_in": w,
         "cst_in": cst}
        for c in range(NCORES)
    ]


def assemble_output(results):
    """Per-core device outputs -> full (2N,) float32 array."""
    dxs = np.stack([np.asarray(results[c]["dx_out"]).astype(np.float32)
                    for c in range(NCORES)])
    dps = np.stack([np.asarray(results[c]["dp_out"]).astype(np.float32)
                    for c in range(NCORES)])
    dx3 = _unshard_compact(dxs)
    dp3 = _unshard_compact(dps)
    return np.concatenate([dx3.reshape(-1), dp3.reshape(-1)])


def kernel(t, y, J, anisotropy, gamma, h_dis_x, h_dis_y, beta, e_disorder,
           nn_idx_1, nn_idx_2, nn_idy_1, nn_idy_2, nn_idz_1, nn_idz_2):
    y = np.asarray(y, np.float32)
    J = np.asarray(J, np.float32)
    anisotropy = np.asarray(anisotropy, np.float32)
    gamma = np.asarray(gamma, np.float32)
    beta = np.asarray(beta, np.float32)
    h_dis_x = np.asarray(h_dis_x, np.float32)
    h_dis_y = np.asarray(h_dis_y, np.float32)
    e_disorder = np.asarray(e_disorder, np.float32)

    ok = (y.shape == (2 * N,)
          and _is_const(J) and _is_const(anisotropy)
          and _is_const(gamma) and _is_const(beta)
          and abs(float(gamma.flat[0])) > 1e-8
          and float(beta.flat[0]) * float(gamma.flat[0]) >= 0
          and _rolls_ok(nn_idx_1, nn_idx_2, nn_idy_1, nn_idy_2,
                        nn_idz_1, nn_idz_2))
    if not ok:
        idx = [np.asarray(a) for a in (nn_idx_1, nn_idx_2, nn_idy_1,
                                       nn_idy_2, nn_idz_1, nn_idz_2)]
        return _numpy_fallback(y, J, anisotropy, gamma, h_dis_x, h_dis_y,
                               beta, e_disorder, idx)

    in_maps = prepare_in_maps(
        y, float(anisotropy.flat[0]), float(gamma.flat[0]),
        float(beta.flat[0]), float(J.flat[0]), h_dis_x, h_dis_y, e_disorder)
    res = _run(in_maps, trace=False)
    return assemble_output(res.results)
